# revision 26
# baseline (speedup 1.0000x reference)
"""Multi-head attention (B=2, S=2048, D=1024, H=16, Dh=64) on 8 trn2 cores.

Sharding: core c = b*4 + g handles batch b and heads [4g, 4g+4).
Per core outputs: attn shard [4, 2048, 2048] f32, unnormalized ctx^T
[2 pairs, 128, 2048] f32 and softmax row sums [128, 4, 16] f32 (host
finishes ctx normalize + transpose, which is tiny).

Single-exp design: scores are computed once per head in [i, j]
orientation (row-packed head pairs on the PE), exp'd on ACT into bf16
panels (with accumulated row sums), normalized to the f32 attn output
on DVE, and the attn@V operand is produced by PE-transposing the bf16
exp panels (16 128x128 transposes per panel pipeline at ~112ns). Each
transposed panel holds the FULL j contraction for one i block, so AV
completes per 4-step group with no long-lived PSUM accumulator.
"""

import numpy as np
import ml_dtypes

import concourse.bass as bass
import concourse.bacc as bacc
import concourse.tile as tile
from concourse import mybir
from concourse.bass_utils import run_bass_kernel_spmd

BF16 = mybir.dt.bfloat16
F32 = mybir.dt.float32
AF = mybir.ActivationFunctionType
ALU = mybir.AluOpType

B, S, D = 2, 2048, 1024
H, DH = 16, 64
HPC = 4          # heads per core
NCORES = 8
SB = S // 128    # 16 s-blocks
KB = D // 128    # 8 contraction blocks

_cache = {}


def _build():
    nc = bacc.Bacc("TRN2", target_bir_lowering=False)

    xq_d = nc.dram_tensor("xq_t", [D, S], BF16, kind="ExternalInput")
    xk_d = nc.dram_tensor("xk_t", [D, S], BF16, kind="ExternalInput")
    xv_d = nc.dram_tensor("xv_t", [D, S], BF16, kind="ExternalInput")
    wq_d = nc.dram_tensor("wq_t", [D, 256], BF16, kind="ExternalInput")
    wk_d = nc.dram_tensor("wk_t", [D, 256], BF16, kind="ExternalInput")
    wv_d = nc.dram_tensor("wv_t", [D, 256], BF16, kind="ExternalInput")
    bq_d = nc.dram_tensor("bq2", [128, 2], F32, kind="ExternalInput")
    bk_d = nc.dram_tensor("bk2", [128, 2], F32, kind="ExternalInput")
    bv_d = nc.dram_tensor("bv_row", [1, 256], BF16, kind="ExternalInput")
    id_d = nc.dram_tensor("ident", [128, 128], BF16, kind="ExternalInput")

    attn_d = nc.dram_tensor("attn_out", [HPC, S, S], F32, kind="ExternalOutput")
    ctx_d = nc.dram_tensor("ctx_out", [2, 128, S], F32, kind="ExternalOutput")
    sums_d = nc.dram_tensor("sums_out", [128, HPC, SB], F32, kind="ExternalOutput")

    with tile.TileContext(nc) as tc:
        with tc.tile_pool(name="persist", bufs=1) as pp:
            q_sb = pp.tile([128, 2, S], BF16, tag="q")   # [p, t, s] qT (+bias, /8)
            k_sb = pp.tile([128, 2, S], BF16, tag="k")   # [p, t, s] kT (+bias)
            v_sb = pp.tile([128, SB, HPC, DH], BF16, tag="v")  # [p, jblk, hh, d]
            sums_sb = pp.tile([128, HPC, SB], F32, tag="sums")
            recip_sb = pp.tile([128, HPC, SB], F32, tag="recip")
            id_sb = pp.tile([128, 128], BF16, tag="ident")
            nc.sync.dma_start(id_sb[:], id_d[:])

            # ---------------- projections (tile 0 + v upfront) ----------
            pw = tc.alloc_tile_pool(name="wts", bufs=1)
            px = tc.alloc_tile_pool(name="xin", bufs=2)
            wq_sb = pw.tile([128, KB, 256], BF16, tag="wq")
            wk_sb = pw.tile([128, KB, 256], BF16, tag="wk")
            wv_sb = pw.tile([128, KB, 256], BF16, tag="wv")
            nc.sync.dma_start(
                wq_sb[:], wq_d[:].rearrange("(kb p) f -> p kb f", p=128))
            nc.sync.dma_start(
                wk_sb[:], wk_d[:].rearrange("(kb p) f -> p kb f", p=128))
            nc.sync.dma_start(
                wv_sb[:], wv_d[:].rearrange("(kb p) f -> p kb f", p=128))
            bq_sb = pw.tile([128, 2], F32, tag="bq")
            bk_sb = pw.tile([128, 2], F32, tag="bk")
            bv_sb = pw.tile([1, 256], BF16, tag="bv")
            nc.sync.dma_start(bq_sb[:], bq_d[:])
            nc.sync.dma_start(bk_sb[:], bk_d[:])
            nc.sync.dma_start(bv_sb[:], bv_d[:])
            ones_sb = pw.tile([1, 128], BF16, tag="ones")
            nc.vector.memset(ones_sb[:], 1.0)

            xq_r = xq_d[:].rearrange("(kb p) s -> p kb s", p=128)
            xk_r = xk_d[:].rearrange("(kb p) s -> p kb s", p=128)
            xv_r = xv_d[:].rearrange("(kb p) s -> p kb s", p=128)

            with tc.tile_pool(name="pproj", bufs=2, space="PSUM") as ppj:
                for sc in range(4):
                    s0 = sc * 512
                    xq_c = px.tile([128, KB, 512], BF16, tag="xq")
                    xk_c = px.tile([128, KB, 512], BF16, tag="xk")
                    xv_c = px.tile([128, KB, 512], BF16, tag="xv")
                    nc.sync.dma_start(xq_c[:], xq_r[:, :, s0:s0 + 512])
                    nc.sync.dma_start(xk_c[:], xk_r[:, :, s0:s0 + 512])
                    nc.sync.dma_start(xv_c[:], xv_r[:, :, s0:s0 + 512])
                    for t in range(2):
                        ps = ppj.tile([128, 512], F32, tag="pq")
                        for kb in range(KB):
                            nc.tensor.matmul(
                                ps[:],
                                wq_sb[:, kb, t * 128:(t + 1) * 128],
                                xq_c[:, kb, :],
                                start=(kb == 0), stop=(kb == KB - 1),
                            )
                        nc.vector.tensor_scalar(
                            q_sb[:, t, s0:s0 + 512], ps[:],
                            bq_sb[:, t:t + 1], 0.125, ALU.add, ALU.mult,
                        )
                        ps2 = ppj.tile([128, 512], F32, tag="pk")
                        for kb in range(KB):
                            nc.tensor.matmul(
                                ps2[:],
                                wk_sb[:, kb, t * 128:(t + 1) * 128],
                                xk_c[:, kb, :],
                                start=(kb == 0), stop=(kb == KB - 1),
                            )
                        nc.vector.tensor_scalar(
                            k_sb[:, t, s0:s0 + 512], ps2[:],
                            bk_sb[:, t:t + 1], None, ALU.add,
                        )
                    for sb4 in range(4):
                        sb = sc * 4 + sb4
                        pv = ppj.tile([128, 256], F32, tag="pv")
                        for kb in range(KB):
                            nc.tensor.matmul(
                                pv[:],
                                xv_c[:, kb, sb4 * 128:(sb4 + 1) * 128],
                                wv_sb[:, kb, :],
                                start=(kb == 0), stop=False,
                            )
                        nc.tensor.matmul(
                            pv[:], ones_sb[:1, :], bv_sb[:1, :],
                            start=False, stop=True,
                        )
                        nc.vector.tensor_copy(
                            v_sb[:, sb, :, :],
                            pv[:].rearrange("p (h d) -> p h d", h=HPC),
                        )
            px.release()
            pw.release()

            # ---------------- attention ----------------
            with (
                tc.tile_pool(name="expb", bufs=5) as pexp,
                tc.tile_pool(name="attn", bufs=5) as pa,
                tc.tile_pool(name="eT4", bufs=2) as pet,
                tc.tile_pool(name="ctxs", bufs=2) as pc,
                tc.tile_pool(name="ppanel", bufs=2, space="PSUM") as ppan,
            ):

                for m in range(2):
                    hA, hB = 2 * m, 2 * m + 1
                    ctx_t = pc.tile([128, S], F32, tag="ctx")
                    eT4_A = None
                    eT4_B = None
                    for ib in range(SB):
                        i0 = ib * 128
                        if ib % 4 == 0:
                            eT4_A = pet.tile([128, SB, 512], BF16, tag="eta")
                            eT4_B = pet.tile([128, SB, 512], BF16, tag="etb")
                        # scores [i, j], row-packed pair
                        oA = ppan.tile([128, S], F32, tag="panel")
                        oB = ppan.tile([128, S], F32, tag="panel")
                        for ic in range(4):
                            c = slice(ic * 512, (ic + 1) * 512)
                            nc.tensor.matmul(
                                oA[:, c], q_sb[0:64, m, i0:i0 + 128],
                                k_sb[0:64, m, c], start=True, stop=True)
                            nc.tensor.matmul(
                                oB[:, c], q_sb[64:128, m, i0:i0 + 128],
                                k_sb[64:128, m, c], start=True, stop=True)
                        # exp (bf16) + row sums; normalize to f32; DMA
                        ebs = []
                        for hh, o in ((hA, oA), (hB, oB)):
                            e = pexp.tile([128, S], BF16, tag="exp")
                            nc.scalar.activation(
                                e[:], o[:], AF.Exp,
                                accum_out=sums_sb[:, hh, ib:ib + 1])
                            nc.vector.reciprocal(
                                recip_sb[:, hh, ib:ib + 1],
                                sums_sb[:, hh, ib:ib + 1])
                            a = pa.tile([128, S], F32, tag="attn")
                            nc.vector.tensor_scalar_mul(
                                a[:], e[:], recip_sb[:, hh, ib:ib + 1])
                            nc.sync.dma_start(
                                attn_d[hh, i0:i0 + 128, :], a[:])
                            ebs.append(e)
                        # transpose exp panels -> [j, i] and stage for AV
                        for e, eT4 in ((ebs[0], eT4_A), (ebs[1], eT4_B)):
                            pt = ppan.tile([128, S], BF16, tag="panel")
                            for cc in range(SB):
                                nc.tensor.transpose(
                                    pt[:, cc * 128:(cc + 1) * 128],
                                    e[:, cc * 128:(cc + 1) * 128],
                                    id_sb[:])
                            nc.vector.tensor_copy(
                                eT4[:, :, (ib % 4) * 128:(ib % 4 + 1) * 128],
                                pt[:].rearrange("p (c u) -> p c u", c=SB))
                        # AV for completed 4-step group
                        if ib % 4 == 3:
                            g = ib // 4
                            cps = ppan.tile([128, 512], F32, tag="panel")
                            for cc in range(SB):
                                st, sp = (cc == 0), (cc == SB - 1)
                                nc.tensor.matmul(
                                    cps[0:64, :], v_sb[:, cc, hA, :],
                                    eT4_A[:, cc, :], start=st, stop=sp,
                                    tile_position=(0, 0))
                                nc.tensor.matmul(
                                    cps[64:128, :], v_sb[:, cc, hB, :],
                                    eT4_B[:, cc, :], start=st, stop=sp,
                                    tile_position=(0, 64))
                            nc.vector.tensor_copy(
                                ctx_t[:, g * 512:(g + 1) * 512], cps[:])
                    nc.sync.dma_start(ctx_d[m], ctx_t[:])
            nc.sync.dma_start(sums_d[:], sums_sb[:])

    nc.finalize()
    return nc


def _prep_core(c, query, key, value, Wq, bq, Wk, bk, Wv, bv):
    b, g = divmod(c, 4)
    h0 = 4 * g
    f0, f1 = h0 * DH, (h0 + HPC) * DH
    bf = ml_dtypes.bfloat16
    return {
        "xq_t": np.ascontiguousarray(query[b].T).astype(bf),
        "xk_t": np.ascontiguousarray(key[b].T).astype(bf),
        "xv_t": np.ascontiguousarray(value[b].T).astype(bf),
        "wq_t": np.ascontiguousarray(Wq[f0:f1].T).astype(bf),
        "wk_t": np.ascontiguousarray(Wk[f0:f1].T).astype(bf),
        "wv_t": np.ascontiguousarray(Wv[f0:f1].T).astype(bf),
        "bq2": np.ascontiguousarray(
            np.asarray(bq[f0:f1], np.float32).reshape(2, 128).T),
        "bk2": np.ascontiguousarray(
            np.asarray(bk[f0:f1], np.float32).reshape(2, 128).T),
        "bv_row": np.asarray(bv[f0:f1], np.float32).reshape(1, 256).astype(bf),
        "ident": np.eye(128).astype(bf),
    }


def kernel(query, key, value, Wq, bq, Wk, bk, Wv, bv, _trace=False):
    query = np.asarray(query, np.float32)
    key = np.asarray(key, np.float32)
    value = np.asarray(value, np.float32)
    Wq, Wk, Wv = (np.asarray(x, np.float32) for x in (Wq, Wk, Wv))
    bq, bk, bv = (np.asarray(x, np.float32) for x in (bq, bk, bv))

    if "nc" not in _cache:
        _cache["nc"] = _build()
    nc = _cache["nc"]

    in_maps = [
        _prep_core(c, query, key, value, Wq, bq, Wk, bk, Wv, bv)
        for c in range(NCORES)
    ]
    res = run_bass_kernel_spmd(nc, in_maps, list(range(NCORES)), trace=_trace)
    _cache["last_results"] = res

    attn_full = np.empty((H * B, S, S), np.float32)
    ctx_full = np.empty((B, S, H * DH), np.float32)
    for c in range(NCORES):
        b, g = divmod(c, 4)
        r = res.results[c]
        sums = r["sums_out"]          # [128, HPC, SB]
        ctxT = r["ctx_out"]           # [2, 128, S] pair-major, unnormalized
        attn = r["attn_out"]          # [HPC, S, S]
        for hh in range(HPC):
            h = 4 * g + hh
            attn_full[2 * h + b] = attn[hh]
            s = sums[:, hh, :].T.reshape(S)     # s[i], i = ib*128 + p
            ct = ctxT[hh // 2, (hh % 2) * 64:(hh % 2) * 64 + 64, :]
            ctx_full[b, :, h * DH:(h + 1) * DH] = (ct / s[None, :]).T
    return ctx_full, attn_full


# revision 29
# speedup vs baseline: 1.0933x; 1.0933x over previous
"""Multi-head attention (B=2, S=2048, D=1024, H=16, Dh=64) on 8 trn2 cores.

Sharding: core c = b*4 + g handles batch b and heads [4g, 4g+4).
Per core outputs: attn shard [4, 2048, 2048] f32, unnormalized ctx^T
[2 pairs, 128, 2048] f32 and softmax row sums [128, 4, 16] f32 (host
finishes ctx normalize + transpose, which is tiny).

Single-exp design: scores are computed once per head in [i, j]
orientation (row-packed head pairs on the PE), exp'd on ACT into bf16
panels (with accumulated row sums), normalized to the f32 attn output
on DVE, and the attn@V operand is produced by PE-transposing the bf16
exp panels (16 128x128 transposes per panel pipeline at ~112ns). Each
transposed panel holds the FULL j contraction for one i block, so AV
completes per 4-step group with no long-lived PSUM accumulator.
"""

import numpy as np
import ml_dtypes

import concourse.bass as bass
import concourse.bacc as bacc
import concourse.tile as tile
from concourse import mybir
from concourse.bass_utils import run_bass_kernel_spmd

BF16 = mybir.dt.bfloat16
F32 = mybir.dt.float32
AF = mybir.ActivationFunctionType
ALU = mybir.AluOpType

B, S, D = 2, 2048, 1024
H, DH = 16, 64
HPC = 4          # heads per core
NCORES = 8
SB = S // 128    # 16 s-blocks
KB = D // 128    # 8 contraction blocks

_cache = {}


def _build():
    nc = bacc.Bacc("TRN2", target_bir_lowering=False)

    xq_d = nc.dram_tensor("xq_t", [D, S], BF16, kind="ExternalInput")
    xk_d = nc.dram_tensor("xk_t", [D, S], BF16, kind="ExternalInput")
    xv_d = nc.dram_tensor("xv_t", [D, S], BF16, kind="ExternalInput")
    wq_d = nc.dram_tensor("wq_t", [D, 256], BF16, kind="ExternalInput")
    wk_d = nc.dram_tensor("wk_t", [D, 256], BF16, kind="ExternalInput")
    wv_d = nc.dram_tensor("wv_t", [D, 256], BF16, kind="ExternalInput")
    bq_d = nc.dram_tensor("bq2", [128, 2], F32, kind="ExternalInput")
    bk_d = nc.dram_tensor("bk2", [128, 2], F32, kind="ExternalInput")
    bv_d = nc.dram_tensor("bv_row", [1, 256], BF16, kind="ExternalInput")
    id_d = nc.dram_tensor("ident", [128, 128], BF16, kind="ExternalInput")

    attn_d = nc.dram_tensor("attn_out", [HPC, S, S], F32, kind="ExternalOutput")
    ctx_d = nc.dram_tensor("ctx_out", [2, 128, S], F32, kind="ExternalOutput")
    sums_d = nc.dram_tensor("sums_out", [128, HPC, SB], F32, kind="ExternalOutput")

    with tile.TileContext(nc) as tc:
        with tc.tile_pool(name="persist", bufs=1) as pp:
            q_sb = pp.tile([128, 2, S], BF16, tag="q")   # [p, t, s] qT (+bias, /8)
            k_sb = pp.tile([128, 2, S], BF16, tag="k")   # [p, t, s] kT (+bias)
            v_sb = pp.tile([128, SB, HPC, DH], BF16, tag="v")  # [p, jblk, hh, d]
            sums_sb = pp.tile([128, HPC, SB], F32, tag="sums")
            sums2_sb = pp.tile([128, HPC, SB], F32, tag="sums2")
            recip_sb = pp.tile([128, HPC, SB], F32, tag="recip")
            id_sb = pp.tile([128, 128], BF16, tag="ident")
            nc.sync.dma_start(id_sb[:], id_d[:])

            # ---------------- projections (tile 0 + v upfront) ----------
            pw = tc.alloc_tile_pool(name="wts", bufs=1)
            px = tc.alloc_tile_pool(name="xin", bufs=2)
            wq_sb = pw.tile([128, KB, 256], BF16, tag="wq")
            wk_sb = pw.tile([128, KB, 256], BF16, tag="wk")
            wv_sb = pw.tile([128, KB, 256], BF16, tag="wv")
            nc.sync.dma_start(
                wq_sb[:], wq_d[:].rearrange("(kb p) f -> p kb f", p=128))
            nc.sync.dma_start(
                wk_sb[:], wk_d[:].rearrange("(kb p) f -> p kb f", p=128))
            nc.sync.dma_start(
                wv_sb[:], wv_d[:].rearrange("(kb p) f -> p kb f", p=128))
            bq_sb = pw.tile([128, 2], F32, tag="bq")
            bk_sb = pw.tile([128, 2], F32, tag="bk")
            bv_sb = pw.tile([1, 256], BF16, tag="bv")
            nc.sync.dma_start(bq_sb[:], bq_d[:])
            nc.sync.dma_start(bk_sb[:], bk_d[:])
            nc.sync.dma_start(bv_sb[:], bv_d[:])
            ones_sb = pw.tile([1, 128], BF16, tag="ones")
            nc.vector.memset(ones_sb[:], 1.0)

            xq_r = xq_d[:].rearrange("(kb p) s -> p kb s", p=128)
            xk_r = xk_d[:].rearrange("(kb p) s -> p kb s", p=128)
            xv_r = xv_d[:].rearrange("(kb p) s -> p kb s", p=128)

            with tc.tile_pool(name="pproj", bufs=2, space="PSUM") as ppj:
                for sc in range(4):
                    s0 = sc * 512
                    xq_c = px.tile([128, KB, 512], BF16, tag="xq")
                    xk_c = px.tile([128, KB, 512], BF16, tag="xk")
                    xv_c = px.tile([128, KB, 512], BF16, tag="xv")
                    nc.sync.dma_start(xq_c[:], xq_r[:, :, s0:s0 + 512])
                    nc.sync.dma_start(xk_c[:], xk_r[:, :, s0:s0 + 512])
                    nc.sync.dma_start(xv_c[:], xv_r[:, :, s0:s0 + 512])
                    for t in range(2):
                        ps = ppj.tile([128, 512], F32, tag="pq")
                        for kb in range(KB):
                            nc.tensor.matmul(
                                ps[:],
                                wq_sb[:, kb, t * 128:(t + 1) * 128],
                                xq_c[:, kb, :],
                                start=(kb == 0), stop=(kb == KB - 1),
                            )
                        nc.vector.tensor_scalar(
                            q_sb[:, t, s0:s0 + 512], ps[:],
                            bq_sb[:, t:t + 1], 0.125, ALU.add, ALU.mult,
                        )
                        ps2 = ppj.tile([128, 512], F32, tag="pk")
                        for kb in range(KB):
                            nc.tensor.matmul(
                                ps2[:],
                                wk_sb[:, kb, t * 128:(t + 1) * 128],
                                xk_c[:, kb, :],
                                start=(kb == 0), stop=(kb == KB - 1),
                            )
                        nc.vector.tensor_scalar(
                            k_sb[:, t, s0:s0 + 512], ps2[:],
                            bk_sb[:, t:t + 1], None, ALU.add,
                        )
                    for sb4 in range(4):
                        sb = sc * 4 + sb4
                        pv = ppj.tile([128, 256], F32, tag="pv")
                        for kb in range(KB):
                            nc.tensor.matmul(
                                pv[:],
                                xv_c[:, kb, sb4 * 128:(sb4 + 1) * 128],
                                wv_sb[:, kb, :],
                                start=(kb == 0), stop=False,
                            )
                        nc.tensor.matmul(
                            pv[:], ones_sb[:1, :], bv_sb[:1, :],
                            start=False, stop=True,
                        )
                        nc.vector.tensor_copy(
                            v_sb[:, sb, :, :],
                            pv[:].rearrange("p (h d) -> p h d", h=HPC),
                        )
            px.release()
            pw.release()

            # ---------------- attention ----------------
            with (
                tc.tile_pool(name="expb", bufs=5) as pexp,
                tc.tile_pool(name="attn", bufs=5) as pa,
                tc.tile_pool(name="eT4", bufs=2) as pet,
                tc.tile_pool(name="ctxs", bufs=2) as pc,
                tc.tile_pool(name="ppanel", bufs=4, space="PSUM") as ppan,
            ):

                for m in range(2):
                    hA, hB = 2 * m, 2 * m + 1
                    ctx_t = pc.tile([128, S], F32, tag="ctx")
                    eT4_A = None
                    eT4_B = None
                    for ib in range(SB):
                        i0 = ib * 128
                        if ib % 4 == 0:
                            eT4_A = pet.tile([128, SB, 512], BF16, tag="eta")
                            eT4_B = pet.tile([128, SB, 512], BF16, tag="etb")
                        # scores [i, j] in half-width panels (4 PSUM
                        # slots), row-packed pair; exp per half with
                        # accumulated partial sums
                        ebs = []
                        for hh, p0 in ((hA, 0), (hB, 64)):
                            e = pexp.tile([128, S], BF16, tag="exp")
                            for half in range(2):
                                o = ppan.tile([128, 1024], F32, tag="panel")
                                for ic2 in range(2):
                                    jc = half * 1024 + ic2 * 512
                                    nc.tensor.matmul(
                                        o[:, ic2 * 512:(ic2 + 1) * 512],
                                        q_sb[p0:p0 + 64, m, i0:i0 + 128],
                                        k_sb[p0:p0 + 64, m, jc:jc + 512],
                                        start=True, stop=True)
                                acc = (sums_sb if half == 0 else
                                       sums2_sb)[:, hh, ib:ib + 1]
                                nc.scalar.activation(
                                    e[:, half * 1024:(half + 1) * 1024],
                                    o[:], AF.Exp, accum_out=acc)
                            nc.vector.tensor_add(
                                sums_sb[:, hh, ib:ib + 1],
                                sums_sb[:, hh, ib:ib + 1],
                                sums2_sb[:, hh, ib:ib + 1])
                            nc.vector.reciprocal(
                                recip_sb[:, hh, ib:ib + 1],
                                sums_sb[:, hh, ib:ib + 1])
                            a = pa.tile([128, S], F32, tag="attn")
                            nc.vector.tensor_scalar_mul(
                                a[:], e[:], recip_sb[:, hh, ib:ib + 1])
                            nc.sync.dma_start(
                                attn_d[hh, i0:i0 + 128, :], a[:])
                            ebs.append(e)
                        # transpose exp panels -> [j, i] and stage for AV
                        for e, eT4 in ((ebs[0], eT4_A), (ebs[1], eT4_B)):
                            for half in range(2):
                                pt = ppan.tile([128, 1024], BF16, tag="panel")
                                for cc in range(8):
                                    cg = half * 8 + cc
                                    nc.tensor.transpose(
                                        pt[:, cc * 128:(cc + 1) * 128],
                                        e[:, cg * 128:(cg + 1) * 128],
                                        id_sb[:])
                                nc.vector.tensor_copy(
                                    eT4[:, half * 8:(half + 1) * 8,
                                        (ib % 4) * 128:(ib % 4 + 1) * 128],
                                    pt[:].rearrange("p (c u) -> p c u", c=8))
                        # AV for completed 4-step group
                        if ib % 4 == 3:
                            g = ib // 4
                            cps = ppan.tile([128, 512], F32, tag="panel")
                            for cc in range(SB):
                                st, sp = (cc == 0), (cc == SB - 1)
                                nc.tensor.matmul(
                                    cps[0:64, :], v_sb[:, cc, hA, :],
                                    eT4_A[:, cc, :], start=st, stop=sp,
                                    tile_position=(0, 0))
                                nc.tensor.matmul(
                                    cps[64:128, :], v_sb[:, cc, hB, :],
                                    eT4_B[:, cc, :], start=st, stop=sp,
                                    tile_position=(0, 64))
                            nc.vector.tensor_copy(
                                ctx_t[:, g * 512:(g + 1) * 512], cps[:])
                    nc.sync.dma_start(ctx_d[m], ctx_t[:])
            nc.sync.dma_start(sums_d[:], sums_sb[:])

    nc.finalize()
    return nc


def _prep_core(c, query, key, value, Wq, bq, Wk, bk, Wv, bv):
    b, g = divmod(c, 4)
    h0 = 4 * g
    f0, f1 = h0 * DH, (h0 + HPC) * DH
    bf = ml_dtypes.bfloat16
    return {
        "xq_t": np.ascontiguousarray(query[b].T).astype(bf),
        "xk_t": np.ascontiguousarray(key[b].T).astype(bf),
        "xv_t": np.ascontiguousarray(value[b].T).astype(bf),
        "wq_t": np.ascontiguousarray(Wq[f0:f1].T).astype(bf),
        "wk_t": np.ascontiguousarray(Wk[f0:f1].T).astype(bf),
        "wv_t": np.ascontiguousarray(Wv[f0:f1].T).astype(bf),
        "bq2": np.ascontiguousarray(
            np.asarray(bq[f0:f1], np.float32).reshape(2, 128).T),
        "bk2": np.ascontiguousarray(
            np.asarray(bk[f0:f1], np.float32).reshape(2, 128).T),
        "bv_row": np.asarray(bv[f0:f1], np.float32).reshape(1, 256).astype(bf),
        "ident": np.eye(128).astype(bf),
    }


def kernel(query, key, value, Wq, bq, Wk, bk, Wv, bv, _trace=False):
    query = np.asarray(query, np.float32)
    key = np.asarray(key, np.float32)
    value = np.asarray(value, np.float32)
    Wq, Wk, Wv = (np.asarray(x, np.float32) for x in (Wq, Wk, Wv))
    bq, bk, bv = (np.asarray(x, np.float32) for x in (bq, bk, bv))

    if "nc" not in _cache:
        _cache["nc"] = _build()
    nc = _cache["nc"]

    in_maps = [
        _prep_core(c, query, key, value, Wq, bq, Wk, bk, Wv, bv)
        for c in range(NCORES)
    ]
    res = run_bass_kernel_spmd(nc, in_maps, list(range(NCORES)), trace=_trace)
    _cache["last_results"] = res

    attn_full = np.empty((H * B, S, S), np.float32)
    ctx_full = np.empty((B, S, H * DH), np.float32)
    for c in range(NCORES):
        b, g = divmod(c, 4)
        r = res.results[c]
        sums = r["sums_out"]          # [128, HPC, SB]
        ctxT = r["ctx_out"]           # [2, 128, S] pair-major, unnormalized
        attn = r["attn_out"]          # [HPC, S, S]
        for hh in range(HPC):
            h = 4 * g + hh
            attn_full[2 * h + b] = attn[hh]
            s = sums[:, hh, :].T.reshape(S)     # s[i], i = ib*128 + p
            ct = ctxT[hh // 2, (hh % 2) * 64:(hh % 2) * 64 + 64, :]
            ctx_full[b, :, h * DH:(h + 1) * DH] = (ct / s[None, :]).T
    return ctx_full, attn_full
